# revision 21
# baseline (speedup 1.0000x reference)
"""BitLinear (input-RMSNorm + ternary-quantized linear) on 8 TRN2 NeuronCores.

Math (reference):
  xn    = x * rsqrt(mean(x^2, -1) + eps) * g
  w     = weight * rsqrt(mean(weight^2, 1) + eps)          (row RMS norm)
  am    = mean(|w|, 1)
  w_q   = sign(w) * (|w| > 0.5*am)                          (ternary)
  out   = xn @ (w_q * am * row_scale).T + bias

Kernel strategy (per core, data-parallel over B*S rows; weight replicated):
  - Single-pass bf16 matmul: x^T rounded to bf16 (~2e-3 rel err vs the
    2e-2 gate); the weight is built ON CHIP as alpha-scaled ternary
    {-alpha, 0, +alpha} (alpha = am*rw*row_scale, folded in during
    ternarization at zero extra cost via two-op tensor_scalar chains),
    so the epilogue is a single STT (pm*sclx)+bias that writes the bf16
    output tile directly.
  - The row rsqrt of x commutes with the matmul (applied per-partition
    to the output); g rides the x^T PSUM->SBUF copies as a per-partition
    scale, split across the scalar and vector engines.
  - |w| > 0.5*mean|w| is evaluated in the raw-weight domain (the rsqrt
    factor cancels); for the fixed benchmark data the smallest relative
    margin to the threshold is 5.4e-7, far above the ~2e-7 rounding
    differences vs the reference, so no mask flips.
  - Software-pipelined skew: 6 weight tiles prep before the main loop,
    the rest pace 2 per s-tile; each (s-tile, chunk) unit is gated only
    on the 4 preps its output columns need, so weight prep hides under
    the PE-bound main loop instead of serializing ahead of it.
  - DMA: x+w loads on the sync HWDGE ring; per-chunk bf16 stores issue
    from the gpsimd queue.
"""

import sys

try:
    import concourse.bass  # noqa: F401
except ImportError:
    for _p in ("/opt/trn_rl_repo", "/root/.axon_site/_ro/trn_rl_repo"):
        if _p not in sys.path:
            sys.path.insert(0, _p)

from contextlib import ExitStack

import numpy as np

import concourse.bass as bass
import concourse.mybir as mybir
import concourse.tile as tile
from concourse import bacc, bass_utils
from concourse.masks import make_identity

B, S, DIN, DOUT = 4, 4096, 2048, 2048
NCORES = 8
SC = B * S // NCORES      # 2048 rows of x per core
P = 128
KT = DIN // P             # 16 k-tiles
ST = SC // P              # 16 s-tiles per core
CH = 512                  # psum chunk (one bank of fp32)
NCH = DOUT // CH          # 4 chunks
EPS = 1e-8
EHEAD = 6                 # weight tiles prepped before the main loop
RPACE = 2                 # weight tiles prepped per early main-loop step

f32 = mybir.dt.float32
f32r = mybir.dt.float32r
bf16 = mybir.dt.bfloat16
AF = mybir.ActivationFunctionType
OP = mybir.AluOpType
AX = mybir.AxisListType


def _skew_schedule():
    """Greedy (tile, chunk) unit order: chunk c is eligible once its 4
    preps are done; units process oldest-tile-first, <=4 per step."""
    steps = []
    pend = []
    npreps = EHEAD
    arrived = 0
    for s in range(ST + 2):
        while arrived < ST and arrived <= s + 1:
            pend += [(arrived, c) for c in range(NCH)]
            arrived += 1
        elig = sorted(u for u in pend if NCH * (u[1] + 1) <= npreps and u[0] <= s)
        take = elig[:NCH]
        for u in take:
            pend.remove(u)
        steps.append(take)
        npreps = min(KT, npreps + RPACE)
    assert not pend, pend
    return steps


def _fr_lifetimes(steps):
    first_use, last_use = {}, {}
    for s, us in enumerate(steps):
        for t, _ in us:
            first_use.setdefault(t, s)
            last_use[t] = s
    alive = max(
        sum(1 for t in first_use if first_use[t] <= s <= last_use[t])
        for s in range(len(steps))
    )
    return first_use, last_use, alive


def build_module(reps=1):
    nc = bacc.Bacc("TRN2", target_bir_lowering=False)
    x_d = nc.declare_dram_parameter("x", [SC, DIN], f32, isOutput=False)
    w_d = nc.declare_dram_parameter("weight", [DOUT, DIN], f32, isOutput=False)
    rs_d = nc.declare_dram_parameter("row_scale", [DOUT, 1], f32, isOutput=False)
    b_d = nc.declare_dram_parameter("bias", [DOUT], f32, isOutput=False)
    g_d = nc.declare_dram_parameter("g", [DIN], f32, isOutput=False)
    o_d = nc.declare_dram_parameter("out", [SC, DOUT], bf16, isOutput=True)

    with tile.TileContext(nc) as tc, ExitStack() as ctx:
        const = ctx.enter_context(tc.tile_pool(name="const", bufs=1))
        xtp = ctx.enter_context(tc.tile_pool(name="xtp", bufs=3))
        wtp = ctx.enter_context(tc.tile_pool(name="wtp", bufs=2))
        atp = ctx.enter_context(tc.tile_pool(name="atp", bufs=2))
        epp = ctx.enter_context(tc.tile_pool(name="epp", bufs=2))
        xgp = ctx.enter_context(tc.tile_pool(name="xgp", bufs=2))
        steps = _skew_schedule()
        first_use, last_use, alive = _fr_lifetimes(steps)
        hip = ctx.enter_context(tc.tile_pool(name="hip", bufs=alive + 1))
        outp = ctx.enter_context(tc.tile_pool(name="outp", bufs=3))
        smp = ctx.enter_context(tc.tile_pool(name="smp", bufs=4))
        pmm = ctx.enter_context(tc.tile_pool(name="pmm", bufs=5, space="PSUM"))
        ptp = ctx.enter_context(tc.tile_pool(name="ptp", bufs=2, space="PSUM"))

        # ---- constants ----
        w2 = const.tile([P, KT, DOUT], bf16)       # alpha-scaled ternary, [i, o]
        bias_b = const.tile([P, DOUT], f32)        # bias broadcast to all partitions
        ident32 = const.tile([P, P], f32)
        identbf = const.tile([P, P], bf16)
        make_identity(nc, ident32)
        nc.vector.tensor_copy(identbf, ident32)
        eps_t = const.tile([P, 1], f32)
        nc.vector.memset(eps_t, EPS)
        g_row = const.tile([P, DIN], f32)          # g broadcast to all partitions
        g_ap = g_d[:]
        nc.gpsimd.dma_start(
            out=g_row,
            in_=bass.AP(
                tensor=g_ap.tensor, offset=g_ap.offset,
                ap=[[0, P]] + list(g_ap.ap),
            ),
        )
        rs_sb = const.tile([P, KT], f32)           # row_scale[o], o = j*128+p
        nc.gpsimd.dma_start(
            out=rs_sb, in_=rs_d.rearrange("(j p) one -> p (j one)", p=P)
        )
        # per-w-tile stats, column j = o-tile j
        sabs = const.tile([P, KT], f32)
        rw = const.tile([P, KT], f32)
        traw = const.tile([P, KT], f32)
        alpha_c = const.tile([P, KT], f32)

        # bias broadcast: DRAM [DOUT] replicated over 128 partitions
        bias_ap = b_d[:]
        nc.gpsimd.dma_start(
            out=bias_b,
            in_=bass.AP(
                tensor=bias_ap.tensor, offset=bias_ap.offset,
                ap=[[0, P]] + list(bias_ap.ap),
            ),
        )

        # ---- weight prep: stats -> scaled ternary -> transpose into w2 ----
        def prep_tile(j):
            w_t = wtp.tile([P, DIN], f32, name="wt")
            nc.sync.dma_start(out=w_t, in_=w_d[j * P : (j + 1) * P, :])
            scr4 = smp.tile([P, 4], f32, name="scr4")

            # ss = sum(w^2) over free dim (scalar engine, accumulator out)
            for c in range(4):
                dump = pmm.tile([P, CH], f32, name="dump", bufs=1)
                nc.scalar.activation(
                    dump, w_t[:, c * CH : (c + 1) * CH], AF.Square,
                    accum_out=scr4[:, c : c + 1],
                )
            nc.vector.tensor_tensor(
                scr4[:, 0:1], scr4[:, 0:1], scr4[:, 1:2], op=OP.add
            )
            nc.vector.tensor_tensor(
                scr4[:, 2:3], scr4[:, 2:3], scr4[:, 3:4], op=OP.add
            )
            nc.vector.tensor_tensor(
                scr4[:, 0:1], scr4[:, 0:1], scr4[:, 2:3], op=OP.add
            )
            # rw_j = sqrt(ss/DIN + eps) ; then reciprocal in place
            nc.scalar.activation(
                rw[:, j : j + 1], scr4[:, 0:1], AF.Sqrt,
                bias=eps_t, scale=1.0 / DIN,
            )
            nc.vector.reciprocal(rw[:, j : j + 1], rw[:, j : j + 1])
            # sumabs = sum(|w|) on the vector engine
            nc.vector.tensor_reduce(
                sabs[:, j : j + 1], w_t, axis=AX.X, op=OP.add,
                apply_absolute_value=True,
            )
            # threshold in the raw-weight domain: traw = 0.5*mean|w|
            nc.vector.tensor_scalar(
                traw[:, j : j + 1], sabs[:, j : j + 1], 0.5 / DIN, None, op0=OP.mult
            )
            # alpha = mean|w| * rw * row_scale
            nc.vector.tensor_scalar(
                scr4[:, 1:2], sabs[:, j : j + 1], 1.0 / DIN, None, op0=OP.mult
            )
            nc.vector.tensor_tensor(
                scr4[:, 1:2], scr4[:, 1:2], rw[:, j : j + 1], op=OP.mult
            )
            nc.vector.tensor_tensor(
                alpha_c[:, j : j + 1], scr4[:, 1:2], rs_sb[:, j : j + 1],
                op=OP.mult,
            )
            # ntraw into scr4[:,3]
            nc.vector.tensor_scalar(
                scr4[:, 3:4], sabs[:, j : j + 1], -0.5 / DIN, None, op0=OP.mult
            )
            # scaled ternary in TWO two-op passes + subtract:
            #   a = (w > traw) * alpha      (gpsimd)
            #   b = (w < -traw) * alpha     (vector)
            #   e = a - b                   (vector)  in {-alpha, 0, +alpha}
            a_t = atp.tile([P, DIN], bf16, name="at")
            nc.gpsimd.tensor_scalar(
                a_t, w_t, traw[:, j : j + 1], alpha_c[:, j : j + 1],
                op0=OP.is_gt, op1=OP.mult,
            )
            b_t = atp.tile([P, DIN], bf16, name="bt")
            nc.vector.tensor_scalar(
                b_t, w_t, scr4[:, 3:4], alpha_c[:, j : j + 1],
                op0=OP.is_lt, op1=OP.mult,
            )
            e_t = epp.tile([P, DIN], bf16, name="et")
            nc.vector.tensor_tensor(e_t, a_t, b_t, op=OP.subtract)
            # transpose e into w2[:, :, j*128:(j+1)*128] (bf16, 1 cyc/row);
            # wide PSUM->SBUF copies alternate scalar/vector engines
            for kk in range(KT // 4):
                pt = ptp.tile([P, 4, P], bf16, name="pt")
                for q in range(4):
                    k = kk * 4 + q
                    nc.tensor.transpose(
                        pt[:, q, :], e_t[:, k * P : (k + 1) * P], identbf
                    )
                dst = w2[:, kk * 4 : kk * 4 + 4, j * P : (j + 1) * P]
                if kk % 2 == 0:
                    nc.scalar.activation(dst, pt, AF.Copy)
                else:
                    nc.vector.tensor_copy(dst, pt)

        # ---- frontend: load x tile t, stats, transpose, g, to bf16 ----
        def frontend(t):
            x_t = xtp.tile([P, DIN], f32, name="xt")
            nc.sync.dma_start(out=x_t, in_=x_d[t * P : (t + 1) * P, :])
            scr4 = smp.tile([P, 4], f32, name="scr4")
            for c in range(4):
                dump = pmm.tile([P, CH], f32, name="dump", bufs=1)
                nc.scalar.activation(
                    dump, x_t[:, c * CH : (c + 1) * CH], AF.Square,
                    accum_out=scr4[:, c : c + 1],
                )
            sclx = smp.tile([P, 1], f32, name="sclx", bufs=8)
            nc.vector.tensor_tensor(
                scr4[:, 0:1], scr4[:, 0:1], scr4[:, 1:2], op=OP.add
            )
            nc.vector.tensor_tensor(
                scr4[:, 2:3], scr4[:, 2:3], scr4[:, 3:4], op=OP.add
            )
            nc.vector.tensor_tensor(
                scr4[:, 0:1], scr4[:, 0:1], scr4[:, 2:3], op=OP.add
            )
            nc.scalar.activation(
                sclx, scr4[:, 0:1], AF.Sqrt, bias=eps_t, scale=1.0 / DIN
            )
            nc.vector.reciprocal(sclx, sclx)

            # x*g in one wide pass (stats above use RAW x); bf16 out makes
            # the transposes 1 cycle/row and the copies 16-bit
            xg_t = xgp.tile([P, DIN], bf16, name="xg")
            nc.vector.tensor_tensor(xg_t, x_t, g_row, op=OP.mult)
            hi_t = hip.tile([P, KT, P], bf16, name="hi")
            for kk in range(KT // 4):
                pt = ptp.tile([P, 4, P], bf16, name="pt")
                for q in range(4):
                    k = kk * 4 + q
                    nc.tensor.transpose(
                        pt[:, q, :], xg_t[:, k * P : (k + 1) * P], identbf
                    )
                nc.vector.tensor_copy(hi_t[:, kk * 4 : kk * 4 + 4, :], pt)
            return hi_t, sclx

        # ---- one (tile, chunk) unit: 16 matmuls + epilogue + store ----
        def unit(t, c, fr):
            hi_t, sclx = fr
            pm = pmm.tile([P, CH], f32, name="pm")
            for k in range(KT):
                nc.tensor.matmul(
                    pm, hi_t[:, k, :], w2[:, k, c * CH : (c + 1) * CH],
                    start=(k == 0), stop=(k == KT - 1),
                )
            obc = outp.tile([P, CH], bf16, name="obc")
            nc.vector.scalar_tensor_tensor(
                out=obc, in0=pm, scalar=sclx,
                in1=bias_b[:, c * CH : (c + 1) * CH],
                op0=OP.mult, op1=OP.add,
            )
            nc.gpsimd.dma_start(
                out=o_d[t * P : (t + 1) * P, c * CH : (c + 1) * CH], in_=obc
            )

        for rep in range(reps):
            fr = {0: frontend(0)}
            for j in range(EHEAD):
                prep_tile(j)
            nprep = EHEAD
            for s, units in enumerate(steps):
                for t, c in units:
                    unit(t, c, fr[t])
                for _ in range(RPACE):
                    if nprep < KT:
                        prep_tile(nprep)
                        nprep += 1
                # just-in-time frontends: emit those first used next step
                for t in range(ST):
                    if t not in fr and first_use.get(t) == s + 1:
                        fr[t] = frontend(t)
                for t in list(fr):
                    if last_use[t] <= s:
                        fr.pop(t)

    nc.compile()
    return nc


_CACHE = {}


def _get_nc():
    if "nc" not in _CACHE:
        _CACHE["nc"] = build_module()
    return _CACHE["nc"]


def kernel(**inputs) -> np.ndarray:
    nc = _get_nc()
    x = np.ascontiguousarray(
        np.asarray(inputs["x"], dtype=np.float32).reshape(B * S, DIN)
    )
    shards = np.split(x, NCORES, axis=0)
    base = {
        k: np.ascontiguousarray(np.asarray(inputs[k], dtype=np.float32))
        for k in ("weight", "row_scale", "bias", "g")
    }
    in_maps = [{"x": shards[c], **base} for c in range(NCORES)]
    res = bass_utils.run_bass_kernel_spmd(nc, in_maps, list(range(NCORES)))
    out = np.concatenate(
        [np.asarray(res.results[c]["out"]) for c in range(NCORES)], axis=0
    )
    return out.reshape(B, S, DOUT).astype(np.float32)


# revision 24
# speedup vs baseline: 3.4640x; 3.4640x over previous
"""BitLinear (input-RMSNorm + ternary-quantized linear) on 8 TRN2 NeuronCores.

Math (reference):
  xn    = x * rsqrt(mean(x^2, -1) + eps) * g
  w     = weight * rsqrt(mean(weight^2, 1) + eps)          (row RMS norm)
  am    = mean(|w|, 1)
  w_q   = sign(w) * (|w| > 0.5*am)                          (ternary)
  out   = xn @ (w_q * am * row_scale).T + bias

Kernel strategy (per core, data-parallel over B*S rows; weight replicated):
  - Single-pass bf16 matmul: x^T rounded to bf16 (~2e-3 rel err vs the
    2e-2 gate); the weight is built ON CHIP as alpha-scaled ternary
    {-alpha, 0, +alpha} (alpha = am*rw*row_scale, folded in during
    ternarization at zero extra cost via two-op tensor_scalar chains),
    so the epilogue is a single STT (pm*sclx)+bias that writes the bf16
    output tile directly.
  - The row rsqrt of x commutes with the matmul (applied per-partition
    to the output); g rides the x^T PSUM->SBUF copies as a per-partition
    scale, split across the scalar and vector engines.
  - |w| > 0.5*mean|w| is evaluated in the raw-weight domain (the rsqrt
    factor cancels); for the fixed benchmark data the smallest relative
    margin to the threshold is 5.4e-7, far above the ~2e-7 rounding
    differences vs the reference, so no mask flips.
  - Software-pipelined skew: 6 weight tiles prep before the main loop,
    the rest pace 2 per s-tile; each (s-tile, chunk) unit is gated only
    on the 4 preps its output columns need, so weight prep hides under
    the PE-bound main loop instead of serializing ahead of it.
  - DMA: x+w loads on the sync HWDGE ring; per-chunk bf16 stores issue
    from the gpsimd queue.
"""

import sys

try:
    import concourse.bass  # noqa: F401
except ImportError:
    for _p in ("/opt/trn_rl_repo", "/root/.axon_site/_ro/trn_rl_repo"):
        if _p not in sys.path:
            sys.path.insert(0, _p)

from contextlib import ExitStack

import numpy as np

import concourse.bass as bass
import concourse.mybir as mybir
import concourse.tile as tile
from concourse import bacc, bass_utils
from concourse.masks import make_identity

B, S, DIN, DOUT = 4, 4096, 2048, 2048
NCORES = 8
SC = B * S // NCORES      # 2048 rows of x per core
P = 128
KT = DIN // P             # 16 k-tiles
ST = SC // P              # 16 s-tiles per core
CH = 512                  # psum chunk (one bank of fp32)
NCH = DOUT // CH          # 4 chunks
EPS = 1e-8
EHEAD = 6                 # weight tiles prepped before the main loop
RPACE = 2                 # weight tiles prepped per early main-loop step

f32 = mybir.dt.float32
f32r = mybir.dt.float32r
bf16 = mybir.dt.bfloat16
AF = mybir.ActivationFunctionType
OP = mybir.AluOpType
AX = mybir.AxisListType


def _skew_schedule():
    """Greedy (tile, chunk) unit order: chunk c is eligible once its 4
    preps are done; units process oldest-tile-first, <=4 per step."""
    steps = []
    pend = []
    npreps = EHEAD
    arrived = 0
    for s in range(ST + 2):
        while arrived < ST and arrived <= s + 1:
            pend += [(arrived, c) for c in range(NCH)]
            arrived += 1
        elig = sorted(u for u in pend if NCH * (u[1] + 1) <= npreps and u[0] <= s)
        take = elig[:NCH]
        for u in take:
            pend.remove(u)
        steps.append(take)
        npreps = min(KT, npreps + RPACE)
    assert not pend, pend
    return steps


def _fr_lifetimes(steps):
    first_use, last_use = {}, {}
    for s, us in enumerate(steps):
        for t, _ in us:
            first_use.setdefault(t, s)
            last_use[t] = s
    alive = max(
        sum(1 for t in first_use if first_use[t] <= s <= last_use[t])
        for s in range(len(steps))
    )
    return first_use, last_use, alive


def build_module(reps=1, store="tile", g_one=False, tern="sign"):
    nc = bacc.Bacc("TRN2", target_bir_lowering=False)
    x_d = nc.declare_dram_parameter("x", [SC, DIN], f32, isOutput=False)
    w_d = nc.declare_dram_parameter("weight", [DOUT, DIN], f32, isOutput=False)
    rs_d = nc.declare_dram_parameter("row_scale", [DOUT, 1], f32, isOutput=False)
    b_d = nc.declare_dram_parameter("bias", [DOUT], f32, isOutput=False)
    g_d = nc.declare_dram_parameter("g", [DIN], f32, isOutput=False)
    o_d = nc.declare_dram_parameter("out", [SC, DOUT], bf16, isOutput=True)

    with tile.TileContext(nc) as tc, ExitStack() as ctx:
        const = ctx.enter_context(tc.tile_pool(name="const", bufs=1))
        xtp = ctx.enter_context(tc.tile_pool(name="xtp", bufs=3))
        wtp = ctx.enter_context(tc.tile_pool(name="wtp", bufs=2))
        atp = ctx.enter_context(tc.tile_pool(name="atp", bufs=2))
        epp = ctx.enter_context(tc.tile_pool(name="epp", bufs=2))
        xgp = ctx.enter_context(tc.tile_pool(name="xgp", bufs=2))
        steps = _skew_schedule()
        first_use, last_use, alive = _fr_lifetimes(steps)
        hip = ctx.enter_context(tc.tile_pool(name="hip", bufs=alive + 1))
        outp = ctx.enter_context(tc.tile_pool(name="outp", bufs=3))
        smp = ctx.enter_context(tc.tile_pool(name="smp", bufs=4))
        pmm = ctx.enter_context(tc.tile_pool(name="pmm", bufs=5, space="PSUM"))
        ptp = ctx.enter_context(tc.tile_pool(name="ptp", bufs=2, space="PSUM"))

        # ---- constants ----
        w2 = const.tile([P, KT, DOUT], bf16)       # alpha-scaled ternary, [i, o]
        bias_b = const.tile([P, DOUT], f32)        # bias broadcast to all partitions
        ident32 = const.tile([P, P], f32)
        identbf = const.tile([P, P], bf16)
        make_identity(nc, ident32)
        nc.vector.tensor_copy(identbf, ident32)
        ident32r = ident32.bitcast(f32r)
        eps_t = const.tile([P, 1], f32)
        nc.vector.memset(eps_t, EPS)
        g_row = const.tile([P, DIN], f32)          # g broadcast to all partitions
        g_ap = g_d[:]
        nc.gpsimd.dma_start(
            out=g_row,
            in_=bass.AP(
                tensor=g_ap.tensor, offset=g_ap.offset,
                ap=[[0, P]] + list(g_ap.ap),
            ),
        )
        rs_sb = const.tile([P, KT], f32)           # row_scale[o], o = j*128+p
        nc.gpsimd.dma_start(
            out=rs_sb, in_=rs_d.rearrange("(j p) one -> p (j one)", p=P)
        )
        # per-w-tile stats, column j = o-tile j
        sabs = const.tile([P, KT], f32)
        rw = const.tile([P, KT], f32)
        traw = const.tile([P, KT], f32)
        alpha_c = const.tile([P, KT], f32)

        # bias broadcast: DRAM [DOUT] replicated over 128 partitions
        bias_ap = b_d[:]
        nc.gpsimd.dma_start(
            out=bias_b,
            in_=bass.AP(
                tensor=bias_ap.tensor, offset=bias_ap.offset,
                ap=[[0, P]] + list(bias_ap.ap),
            ),
        )

        # ---- weight prep: stats -> scaled ternary -> transpose into w2 ----
        def prep_tile(j):
            w_t = wtp.tile([P, DIN], f32, name="wt")
            nc.sync.dma_start(out=w_t, in_=w_d[j * P : (j + 1) * P, :])
            scr4 = smp.tile([P, 4], f32, name="scr4")

            # ss = sum(w^2) over free dim (scalar engine, accumulator out)
            for c in range(4):
                dump = pmm.tile([P, CH], f32, name="dump", bufs=1)
                nc.scalar.activation(
                    dump, w_t[:, c * CH : (c + 1) * CH], AF.Square,
                    accum_out=scr4[:, c : c + 1],
                )
            nc.vector.tensor_tensor(
                scr4[:, 0:1], scr4[:, 0:1], scr4[:, 1:2], op=OP.add
            )
            nc.vector.tensor_tensor(
                scr4[:, 2:3], scr4[:, 2:3], scr4[:, 3:4], op=OP.add
            )
            nc.vector.tensor_tensor(
                scr4[:, 0:1], scr4[:, 0:1], scr4[:, 2:3], op=OP.add
            )
            # rw_j = sqrt(ss/DIN + eps) ; then reciprocal in place
            nc.scalar.activation(
                rw[:, j : j + 1], scr4[:, 0:1], AF.Sqrt,
                bias=eps_t, scale=1.0 / DIN,
            )
            nc.vector.reciprocal(rw[:, j : j + 1], rw[:, j : j + 1])
            # sumabs = sum(|w|) on the vector engine
            nc.vector.tensor_reduce(
                sabs[:, j : j + 1], w_t, axis=AX.X, op=OP.add,
                apply_absolute_value=True,
            )
            # threshold in the raw-weight domain: traw = 0.5*mean|w|
            nc.vector.tensor_scalar(
                traw[:, j : j + 1], sabs[:, j : j + 1], 0.5 / DIN, None, op0=OP.mult
            )
            # alpha = mean|w| * rw * row_scale
            nc.vector.tensor_scalar(
                scr4[:, 1:2], sabs[:, j : j + 1], 1.0 / DIN, None, op0=OP.mult
            )
            nc.vector.tensor_tensor(
                scr4[:, 1:2], scr4[:, 1:2], rw[:, j : j + 1], op=OP.mult
            )
            nc.vector.tensor_tensor(
                alpha_c[:, j : j + 1], scr4[:, 1:2], rs_sb[:, j : j + 1],
                op=OP.mult,
            )
            # ntraw into scr4[:,3]
            nc.vector.tensor_scalar(
                scr4[:, 3:4], sabs[:, j : j + 1], -0.5 / DIN, None, op0=OP.mult
            )
            if tern == "ab":
                # scaled ternary in TWO two-op passes + subtract:
                #   a = (w > traw) * alpha      (gpsimd)
                #   b = (w < -traw) * alpha     (vector)
                #   e = a - b                   (vector)  in {-alpha, 0, +alpha}
                a_t = atp.tile([P, DIN], bf16, name="at")
                nc.gpsimd.tensor_scalar(
                    a_t, w_t, traw[:, j : j + 1], alpha_c[:, j : j + 1],
                    op0=OP.is_gt, op1=OP.mult,
                )
                b_t = atp.tile([P, DIN], bf16, name="bt")
                nc.vector.tensor_scalar(
                    b_t, w_t, scr4[:, 3:4], alpha_c[:, j : j + 1],
                    op0=OP.is_lt, op1=OP.mult,
                )
                e_t = epp.tile([P, DIN], bf16, name="et")
                nc.vector.tensor_tensor(e_t, a_t, b_t, op=OP.subtract)
            else:
                # sign trick on the scalar engine:
                #   s1 = Sign(w - t), s2 = Sign(w + t)  (activation, bias op)
                #   e  = (s1 + s2) * alpha/2            (vector, bf16 rate)
                nc.vector.tensor_scalar(
                    scr4[:, 2:3], alpha_c[:, j : j + 1], 0.5, None, op0=OP.mult
                )
                s1 = atp.tile([P, DIN], bf16, name="at")
                nc.scalar.activation(s1, w_t, AF.Sign, bias=scr4[:, 3:4])
                s2 = atp.tile([P, DIN], bf16, name="bt")
                nc.scalar.activation(s2, w_t, AF.Sign, bias=traw[:, j : j + 1])
                s12 = xgp.tile([P, DIN], bf16, name="s12")
                nc.vector.tensor_tensor(s12, s1, s2, op=OP.add)
                e_t = epp.tile([P, DIN], bf16, name="et")
                nc.vector.tensor_scalar(
                    e_t, s12, scr4[:, 2:3], None, op0=OP.mult
                )
            # transpose e into w2[:, :, j*128:(j+1)*128] (bf16, 1 cyc/row);
            # wide PSUM->SBUF copies alternate scalar/vector engines
            for kk in range(KT // 4):
                pt = ptp.tile([P, 4, P], bf16, name="pt")
                for q in range(4):
                    k = kk * 4 + q
                    nc.tensor.transpose(
                        pt[:, q, :], e_t[:, k * P : (k + 1) * P], identbf
                    )
                dst = w2[:, kk * 4 : kk * 4 + 4, j * P : (j + 1) * P]
                if kk % 2 == 0:
                    nc.scalar.activation(dst, pt, AF.Copy)
                else:
                    nc.vector.tensor_copy(dst, pt)

        # ---- frontend: load x tile t, stats, transpose, g, to bf16 ----
        def frontend(t):
            x_t = xtp.tile([P, DIN], f32, name="xt")
            nc.sync.dma_start(out=x_t, in_=x_d[t * P : (t + 1) * P, :])
            scr4 = smp.tile([P, 4], f32, name="scr4")
            for c in range(4):
                dump = pmm.tile([P, CH], f32, name="dump", bufs=1)
                nc.scalar.activation(
                    dump, x_t[:, c * CH : (c + 1) * CH], AF.Square,
                    accum_out=scr4[:, c : c + 1],
                )
            sclx = smp.tile([P, 1], f32, name="sclx", bufs=8)
            nc.vector.tensor_tensor(
                scr4[:, 0:1], scr4[:, 0:1], scr4[:, 1:2], op=OP.add
            )
            nc.vector.tensor_tensor(
                scr4[:, 2:3], scr4[:, 2:3], scr4[:, 3:4], op=OP.add
            )
            nc.vector.tensor_tensor(
                scr4[:, 0:1], scr4[:, 0:1], scr4[:, 2:3], op=OP.add
            )
            nc.scalar.activation(
                sclx, scr4[:, 0:1], AF.Sqrt, bias=eps_t, scale=1.0 / DIN
            )
            nc.vector.reciprocal(sclx, sclx)

            hi_t = hip.tile([P, KT, P], bf16, name="hi")
            if g_one:
                # g == 1 per the problem spec (fill: ones): transpose x
                # directly in f32r, round to bf16 in the wide copies
                for kk in range(KT // 4):
                    pt = ptp.tile([P, 4, P], f32r, name="pt")
                    for q in range(4):
                        k = kk * 4 + q
                        nc.tensor.transpose(
                            pt[:, q, :],
                            x_t[:, k * P : (k + 1) * P].bitcast(f32r), ident32r,
                        )
                    nc.vector.tensor_copy(hi_t[:, kk * 4 : kk * 4 + 4, :], pt)
                return hi_t, sclx
            # x*g in one wide pass (stats above use RAW x); bf16 out makes
            # the transposes 1 cycle/row and the copies 16-bit
            xg_t = xgp.tile([P, DIN], bf16, name="xg")
            nc.vector.tensor_tensor(xg_t, x_t, g_row, op=OP.mult)
            for kk in range(KT // 4):
                pt = ptp.tile([P, 4, P], bf16, name="pt")
                for q in range(4):
                    k = kk * 4 + q
                    nc.tensor.transpose(
                        pt[:, q, :], xg_t[:, k * P : (k + 1) * P], identbf
                    )
                nc.vector.tensor_copy(hi_t[:, kk * 4 : kk * 4 + 4, :], pt)
            return hi_t, sclx

        # ---- one (tile, chunk) unit: 16 matmuls + epilogue + store ----
        def unit(t, c, fr, ob=None):
            hi_t, sclx = fr
            pm = pmm.tile([P, CH], f32, name="pm")
            for k in range(KT):
                nc.tensor.matmul(
                    pm, hi_t[:, k, :], w2[:, k, c * CH : (c + 1) * CH],
                    start=(k == 0), stop=(k == KT - 1),
                )
            if ob is None:
                obc = outp.tile([P, CH], bf16, name="obc")
            else:
                obc = ob[:, c * CH : (c + 1) * CH]
            nc.vector.scalar_tensor_tensor(
                out=obc, in0=pm, scalar=sclx,
                in1=bias_b[:, c * CH : (c + 1) * CH],
                op0=OP.mult, op1=OP.add,
            )
            if ob is None:
                nc.gpsimd.dma_start(
                    out=o_d[t * P : (t + 1) * P, c * CH : (c + 1) * CH],
                    in_=obc,
                )

        for rep in range(reps):
            fr = {0: frontend(0)}
            for j in range(EHEAD):
                prep_tile(j)
            nprep = EHEAD
            obs, ndone = {}, {}
            for s, units in enumerate(steps):
                for t, c in units:
                    if store == "chunk":
                        unit(t, c, fr[t])
                    else:
                        if t not in obs:
                            obs[t] = outp.tile(
                                [P, DOUT], bf16, name="ob", bufs=5
                            )
                            ndone[t] = 0
                        unit(t, c, fr[t], obs[t])
                        ndone[t] += 1
                        if ndone[t] == NCH:
                            # one merged SWDGE store per s-tile
                            nc.gpsimd.dma_start(
                                out=o_d[t * P : (t + 1) * P, :],
                                in_=obs.pop(t),
                            )
                for _ in range(RPACE):
                    if nprep < KT:
                        prep_tile(nprep)
                        nprep += 1
                # just-in-time frontends: emit those first used next step
                for t in range(ST):
                    if t not in fr and first_use.get(t) == s + 1:
                        fr[t] = frontend(t)
                for t in list(fr):
                    if last_use[t] <= s:
                        fr.pop(t)

    nc.compile()
    return nc


_CACHE = {}


def _get_nc():
    if "nc" not in _CACHE:
        _CACHE["nc"] = build_module()
    return _CACHE["nc"]


def kernel(**inputs) -> np.ndarray:
    nc = _get_nc()
    x = np.ascontiguousarray(
        np.asarray(inputs["x"], dtype=np.float32).reshape(B * S, DIN)
    )
    shards = np.split(x, NCORES, axis=0)
    base = {
        k: np.ascontiguousarray(np.asarray(inputs[k], dtype=np.float32))
        for k in ("weight", "row_scale", "bias", "g")
    }
    in_maps = [{"x": shards[c], **base} for c in range(NCORES)]
    res = bass_utils.run_bass_kernel_spmd(nc, in_maps, list(range(NCORES)))
    out = np.concatenate(
        [np.asarray(res.results[c]["out"]) for c in range(NCORES)], axis=0
    )
    return out.reshape(B, S, DOUT).astype(np.float32)
